# revision 1
# baseline (speedup 1.0000x reference)
"""Trainium2 Bass kernel for MinimalConvWTA_LIF.

Model: u = three causal convs (k=8/16/32, scaled 1/sqrt(k)) over x[B,1,T];
s = winner-take-all LIF spike train over u with alpha=0.95, theta=1.0.

Strategy (per NeuronCore, pure data parallel over batch, 32 rows/core):
  * conv: PE matmuls.  x is transposed into 128-row time tiles via PE
    transpose; each output window of 96 timesteps is one (or two, when the
    window straddles a 128-tile boundary) f32 matmul against a host-built
    banded weight matrix [128, 3*96].
  * LIF scan: the time axis is split into 64 chunks of C=256.  All chunks
    are advanced simultaneously (wavefront): SBUF layout [128 partitions =
    32 batches x 4 chunk-slots, free = 16 chunks x (3 channels + const
    threshold lane)].  One timestep = 4 DVE instructions covering every
    chunk:
       1. v = (v * alpha) + u_t          (scalar_tensor_tensor)
       2. gmax = max(v0,v1,v2,1.0)       (pool_max over the 4-lane group)
       3. s = (v >= gmax)                (tensor_tensor is_ge, broadcast)
       4. v = v - s                      (tensor_tensor subtract)
    The threshold constant 1.0 rides as lane 3 of each group, so (3) is
    exactly "spike iff v == max(v) and v >= theta".
  * chunk boundary states are resolved by iteration: pass 1 starts every
    chunk at v=0; pass p+1 re-runs every chunk initialised with the end
    state of its left neighbour from pass p.  With C=256, 3 passes converge
    exactly (alpha^512 ~ 4e-12 contraction).
"""

import os
import sys

import numpy as np

_TRN_REPO = "/opt/trn_rl_repo"
if _TRN_REPO not in sys.path:
    sys.path.insert(0, _TRN_REPO)

import concourse.bass as bass
import concourse.mybir as mybir
from concourse import bacc, tile
from concourse.bass_utils import run_bass_kernel_spmd

# ---------------------------------------------------------------- constants
B_FULL = 256
T_FULL = 16384
N_CORES = 8
KERNELS = (8, 16, 32)
ALPHA = np.float32(0.95)
F32 = mybir.dt.float32

# conv window geometry: outputs come in 128-aligned blocks.  Block j needs
# padded inputs [128j+97, 128j+256): rows [64,128) of padded tile j (matmul A,
# against a band matrix whose rows 64..96 are structurally zero) plus all of
# padded tile j+1 (matmul B).  x is left-padded by one full 128-zero tile.
WIN_OUT = 128
LPAD = 128


class Cfg:
    def __init__(self, Bc=32, T=16384, C=256, CS=4, P=3):
        self.Bc = Bc          # batch rows per core
        self.T = T
        self.C = C            # chunk length (timesteps)
        self.CS = CS          # chunk slots along partitions
        self.P = P            # boundary-iteration passes
        self.NCH = T // C     # total chunks
        assert self.NCH % CS == 0
        self.NC2 = self.NCH // CS   # chunks along the free dim
        self.NQ = 4                 # step-quarter tiles (pipelining granularity)
        assert C % self.NQ == 0
        self.Q = C // self.NQ
        assert T % 128 == 0
        self.NW = T // 128          # conv output blocks
        self.XTILES = self.NW + 1   # padded x tiles (one leading zero tile)
        self.XP_LEN = 128 * self.XTILES
        assert Bc * CS <= 128


# ------------------------------------------------------------- host helpers
def build_walls(ws):
    """Banded conv-weight matrices wallA, wallB, each [128, 3*128].

    Output block j (tau = 128j + tl, tl in [0,128)) is
        sum_d w_k[kl-1-d] * xp[128j + 128 + tl - d]
      = xT[64:128, tile j].T   @ wallA[64:128]    (d = tl + 128 - r, r>=97)
      + xT[0:128, tile j+1].T  @ wallB            (d = tl - r)
    """
    wallA = np.zeros((128, 3 * 32), np.float32)
    wallB = np.zeros((128, 3 * WIN_OUT), np.float32)
    for k, w in enumerate(ws):
        kl = len(w)
        scale = np.float32(1.0 / np.sqrt(np.float32(kl)))
        wk = (w.astype(np.float32) * scale).astype(np.float32)
        for tl in range(WIN_OUT):
            for d in range(kl):
                rA = tl + 128 - d
                if 64 <= rA < 128 and tl < 32:
                    wallA[rA, tl * 3 + k] = wk[kl - 1 - d]
                rB = tl - d
                if 0 <= rB < 128:
                    wallB[rB, tl * 3 + k] = wk[kl - 1 - d]
    return wallA, wallB


def pad_x(x2d, cfg):
    """[B, T] -> [B, XP_LEN] with LPAD zeros in front."""
    out = np.zeros((x2d.shape[0], cfg.XP_LEN), np.float32)
    out[:, LPAD:LPAD + cfg.T] = x2d
    return out


# ------------------------------------------------------------ program build
def build_program(cfg):
    nc = bacc.Bacc("TRN2", target_bir_lowering=False, debug=False)

    x_d = nc.dram_tensor("x_pad", [cfg.Bc, cfg.XP_LEN], F32, kind="ExternalInput")
    wa_d = nc.dram_tensor("wallA", [128, 3 * 32], F32, kind="ExternalInput")
    wb_d = nc.dram_tensor("wallB", [128, 3 * WIN_OUT], F32, kind="ExternalInput")
    id_d = nc.dram_tensor("ident", [cfg.Bc, cfg.Bc], F32, kind="ExternalInput")
    u_d = nc.dram_tensor("u_out", [cfg.Bc, 3, cfg.T], F32, kind="ExternalOutput")
    s_d = nc.dram_tensor("s_out", [cfg.Bc, 3, cfg.T], F32, kind="ExternalOutput")

    Bc, C, CS, NC2, NQ, Q = cfg.Bc, cfg.C, cfg.CS, cfg.NC2, cfg.NQ, cfg.Q

    with tile.TileContext(nc) as tc:
        with (
            tc.tile_pool(name="const", bufs=1) as constp,
            tc.tile_pool(name="xbuf", bufs=1) as xbuf,
            tc.tile_pool(name="wave", bufs=1) as wave,
            tc.tile_pool(name="state", bufs=1) as state,
            tc.tile_pool(name="psT", bufs=4, space="PSUM") as psT,
            tc.tile_pool(name="psC", bufs=4, space="PSUM") as psC,
        ):
            x_sb = xbuf.tile([Bc, cfg.XP_LEN], F32, tag="x")
            wa_sb = constp.tile([128, 3 * 32], F32, tag="wa")
            wb_sb = constp.tile([128, 3 * WIN_OUT], F32, tag="wb")
            id_sb = constp.tile([Bc, Bc], F32, tag="id")
            # split the x load so the first transposes can start early
            nxd = 8
            assert cfg.XP_LEN % nxd == 0
            xsl = cfg.XP_LEN // nxd
            for i in range(nxd):
                nc.sync.dma_start(x_sb[:, i * xsl:(i + 1) * xsl],
                                  x_d.ap()[:, i * xsl:(i + 1) * xsl])
            nc.sync.dma_start(wa_sb[:], wa_d.ap())
            nc.sync.dma_start(wb_sb[:], wb_d.ap())
            nc.sync.dma_start(id_sb[:], id_d.ap())

            # transposed x strip: [128 (time within tile), XTILES*Bc].
            # Transposes are emitted lazily, interleaved with the conv
            # windows that consume them; the PSUM->SBUF copies ride on the
            # Vector engine, which is otherwise idle until the wavefront.
            # even/odd tile strips keep each window-pack's tiles contiguous
            # (matmul stationary APs must have a single free dimension)
            ne = (cfg.XTILES + 1) // 2
            no = cfg.XTILES // 2
            xTe = xbuf.tile([128, ne, Bc], F32, tag="xTe")
            xTo = xbuf.tile([128, no, Bc], F32, tag="xTo")
            _emitted = set()

            def ensure_xT(j):
                if j in _emitted:
                    return
                _emitted.add(j)
                pt = psT.tile([128, Bc], F32, tag="psT", name=f"psT{j}")
                nc.tensor.transpose(pt[:], x_sb[:, 128 * j:128 * (j + 1)],
                                    id_sb[:])
                strip = xTe if j % 2 == 0 else xTo
                nc.vector.tensor_copy(strip[:, j // 2, :], pt[:])

            def xt_flat(first_tile, ntiles, rows=None):
                strip = xTe if first_tile % 2 == 0 else xTo
                a = strip[:, 0, :] if rows is None else strip[rows[0]:rows[1], 0, :]
                return bass.AP(a.tensor, a.offset + (first_tile // 2) * Bc,
                               [a.ap[0], [1, ntiles * Bc]])

            # u in wavefront layout, quartered along the step axis:
            # uq[q][p=(b + 32*cs), c2, k, jq]   (t = (cs*NC2+c2)*C + q*Q + jq)
            uq = [wave.tile([Bc * CS, NC2, 3, Q], F32, tag=f"uq{q}", name=f"uq{q}")
                  for q in range(NQ)]
            sq = [wave.tile([Bc * CS, NC2, 3, Q], F32, tag=f"sq{q}", name=f"sq{q}")
                  for q in range(NQ)]

            # conv output blocks -> PSUM -> scatter into uq.
            # Early LIF steps need u for EVERY chunk, so produce the first
            # half of every chunk before any second half (even blocks first).
            # PK windows are packed into one matmul pair: each window's
            # transposed-x occupies Bc stationary columns, all sharing the
            # same moving band matrix; output partitions = PK * Bc.
            worder = sorted(range(cfg.NW), key=lambda w: ((WIN_OUT * w) % C, w))
            PK = 128 // Bc
            # per-window matmuls overlap best with the transpose stream
            groups = [[w] for w in worder]
            for grp in groups:
                for w in grp:
                    ensure_xT(w)
                    ensure_xT(w + 1)
                npk = len(grp)
                pc = psC.tile([Bc * npk, WIN_OUT, 3], F32, tag="psC")
                pc_flat = bass.AP(pc[:].tensor, pc[:].offset,
                                  [pc[:].ap[0], [1, 3 * WIN_OUT]])
                pc_head = bass.AP(pc[:].tensor, pc[:].offset,
                                  [pc[:].ap[0], [1, 3 * 32]])
                lhsB = xt_flat(grp[0] + 1, npk)
                lhsA = xt_flat(grp[0], npk, rows=(64, 128))
                nc.tensor.matmul(pc_flat, lhsB, wb_sb[:],
                                 start=True, stop=False)
                nc.tensor.matmul(pc_head, lhsA, wa_sb[64:128, :],
                                 start=False, stop=True)
                for gi, w in enumerate(grp):
                    w0 = WIN_OUT * w
                    pcs = pc[Bc * gi:Bc * (gi + 1), :, :]
                    ta = w0
                    tb = w0 + WIN_OUT
                    while ta < tb:
                        c = ta // C
                        step = ta - c * C
                        q = step // Q
                        jq = step - q * Q
                        run = min(tb - ta, C - step, Q - jq)
                        cs, c2 = c // NC2, c % NC2
                        src_ap = bass.AP(pcs.tensor,
                                         pcs.offset + (ta - w0) * 3,
                                         [pcs.ap[0], [1, 3], [3, run]])
                        nc.scalar.copy(
                            uq[q][Bc * cs:Bc * (cs + 1), c2, :, jq:jq + run],
                            src_ap)
                        ta += run

            # u DMA out: t = (cs*NC2 + c2)*C + q*Q + jq   (one DMA per cs,q,k)
            for cs in range(CS):
                for q in range(NQ):
                    for k in range(3):
                        src = uq[q][Bc * cs:Bc * (cs + 1), :, k, :]
                        dst_ap = bass.AP(
                            u_d.ap().tensor,
                            (k * cfg.T + cs * NC2 * C + q * Q),
                            [[3 * cfg.T, Bc], [C, NC2], [1, Q]])
                        nc.sync.dma_start(dst_ap, src)

            # ------------------------------------------------ LIF wavefront
            va = state.tile([Bc * CS, NC2, 4], F32, tag="va")
            vb = state.tile([Bc * CS, NC2, 4], F32, tag="vb")
            gmax = state.tile([Bc * CS, NC2], F32, tag="gmax")
            g_ap = gmax[:, :]
            gmax_b = bass.AP(g_ap.tensor, g_ap.offset, list(g_ap.ap) + [[0, 3]])

            # lane 3 of each group holds the constant threshold 1.0, so the
            # group max is max(v0,v1,v2,theta) and "spike iff v >= gmax".
            nc.vector.memset(va[:, :, 0:3], 0.0)
            nc.vector.memset(va[:, :, 3:4], 1.0)
            nc.vector.memset(vb[:, :, 3:4], 1.0)

            vtiles = [va, vb]
            for p in range(cfg.P):
                v = vtiles[p % 2]
                if p > 0:
                    vprev = vtiles[(p - 1) % 2]
                    # chunk c starts from end state of chunk c-1 of prev pass
                    nc.vector.tensor_copy(v[:, 1:NC2, :], vprev[:, 0:NC2 - 1, :])
                    for cs in range(1, CS):
                        nc.vector.tensor_copy(
                            v[Bc * cs:Bc * (cs + 1), 0, :],
                            vprev[Bc * (cs - 1):Bc * cs, NC2 - 1, :])
                    nc.vector.memset(v[0:Bc, 0:1, 0:3], 0.0)
                for step in range(C):
                    q, jq = step // Q, step % Q
                    u_sl = uq[q][:, :, :, jq]
                    s_sl = sq[q][:, :, :, jq]
                    nc.vector.scalar_tensor_tensor(
                        v[:, :, 0:3], v[:, :, 0:3], float(ALPHA), u_sl,
                        op0=mybir.AluOpType.mult, op1=mybir.AluOpType.add)
                    nc.vector.tensor_reduce(
                        gmax[:, :], v[:, :, :], axis=mybir.AxisListType.X,
                        op=mybir.AluOpType.max)
                    nc.vector.tensor_tensor(
                        s_sl, v[:, :, 0:3], gmax_b, op=mybir.AluOpType.is_ge)
                    nc.vector.tensor_tensor(
                        v[:, :, 0:3], v[:, :, 0:3], s_sl,
                        op=mybir.AluOpType.subtract)

            # s DMA out
            for cs in range(CS):
                for q in range(NQ):
                    for k in range(3):
                        src = sq[q][Bc * cs:Bc * (cs + 1), :, k, :]
                        dst_ap = bass.AP(
                            s_d.ap().tensor,
                            (k * cfg.T + cs * NC2 * C + q * Q),
                            [[3 * cfg.T, Bc], [C, NC2], [1, Q]])
                        nc.sync.dma_start(dst_ap, src)

    nc.compile()
    return nc


# ----------------------------------------------------------------- running
def _ensure_ntff_hook():
    """Register the axon NTFF profiling hook (the image's antenv lacks the
    axon_hooks registry module; inject it and wire up the ctypes hook)."""
    import types
    try:
        from antenv.axon_hooks import get_axon_ntff_profile_hook  # noqa: F401
        return
    except ImportError:
        pass
    import antenv
    mod = types.ModuleType("antenv.axon_hooks")
    _state = {"hook": None}
    mod.set_axon_ntff_profile_hook = lambda h: _state.__setitem__("hook", h)
    mod.get_axon_ntff_profile_hook = lambda: _state["hook"]
    sys.modules["antenv.axon_hooks"] = mod
    antenv.axon_hooks = mod
    try:
        from trn_agent_boot.trn_boot import _ntff_profile_via_ctypes
        hook = _ntff_profile_via_ctypes("/opt/axon/libaxon_pjrt.so")
        if hook is not None:
            mod.set_axon_ntff_profile_hook(hook)
    except Exception as e:  # profiling optional
        print(f"ntff hook unavailable: {e}", file=sys.stderr)


_CACHE = {}


def _get_program(cfg_key=None):
    if cfg_key not in _CACHE:
        _CACHE[cfg_key] = build_program(Cfg())
    return _CACHE[cfg_key]


def kernel(x, w0, w1, w2, y=None, trace=False):
    x = np.asarray(x, np.float32)
    ws = [np.asarray(w, np.float32).reshape(-1) for w in (w0, w1, w2)]
    cfg = Cfg()
    B = x.shape[0]
    assert B == B_FULL and x.shape[-1] == T_FULL

    wallA, wallB = build_walls(ws)
    ident = np.eye(cfg.Bc, dtype=np.float32)
    xp = pad_x(x.reshape(B, T_FULL), cfg)

    if trace:
        _ensure_ntff_hook()
    nc = _get_program()
    in_maps = [
        {"x_pad": xp[c * cfg.Bc:(c + 1) * cfg.Bc],
         "wallA": wallA, "wallB": wallB, "ident": ident}
        for c in range(N_CORES)
    ]
    res = run_bass_kernel_spmd(nc, in_maps, core_ids=list(range(N_CORES)),
                               trace=trace)
    u = np.concatenate([r["u_out"] for r in res.results], axis=0)
    s = np.concatenate([r["s_out"] for r in res.results], axis=0)
    if trace:
        kernel.last_exec_time_ns = res.exec_time_ns
    return (u, s)


kernel.last_exec_time_ns = None

